# revision 26
# baseline (speedup 1.0000x reference)
"""Trainium2 Bass kernel for nn_DeltaModel (histogram_binning).

Reference semantics (delta == 0, the shipped configuration):
  med[t,ch]   = lower median over N of logits[t,:,ch]          (rows 0-4 used)
  q[n,ch]     = sumsq - 0.1*sum^2  over the 10 rows            (= 9*unbiased var)
  std_med[ch] = sqrt(median_N(q[:,ch]) / 9)
  mode[n,ch]  = (#{t<5: logits[t,n,ch] >= med[t,ch] + 1.96*std_med[ch]} >= 3)
  c           = broadcast(mode) over dim 0
  out[t,:,ch] = xs[t,ch] - logsumexp(xs[t,others(ch)])  (constant over N)

Every step couples only within a channel, so the whole device computation is
ONE SPMD launch on 4 cores, each owning one channel end-to-end (core c gets
the 10 full (t, ch=c) slices, 40MB):
  phase A: stream the 10 rows -> q[n] = sumsq - 0.1*sum^2     (vector engine)
  phase A2: bisection median of q (17 levels) -> qmed; th precursor
            1.96*sqrt(qmed/9) on the Scalar engine
  phase B: bisection medians of rows 0-4 (16 levels, exact to 3e-7)
  phase C: th[t] = med[t] + 1.96*std_med; mode = (#{x >= th[t]} >= 3)
The client->device link (~30 MB/s axon tunnel) dominates wall time, so this
sharding is chosen to minimize shipped bytes: 160MB in + 16MB out, nothing
shipped twice.  Host does only padding, the (10,4) logsumexp table, and
broadcast-view assembly.  Bracket misses (never for iid N(0,1) inputs) fall
back to exact host re-derivation per channel.
"""

import os

# Frame->traceback capture during Bass build bloats per-instruction debug
# info and slows launches by tens of seconds; disable before concourse loads.
os.environ.setdefault("BASS_DISABLE_FRAME_TO_TRACEBACK", "1")

import numpy as np

LAST_RUN_TIMES = []  # wall seconds of each device launch (incl. first-call compile)

N = 1_000_000
NROWS = 10
NCH = 4
SLICE_FREE = 7816              # per-partition elements of a 1M slice
SLICE_PAD = 128 * SLICE_FREE   # 1000448
CHUNK = 977                    # 7816 = 8 * 977
PAD_BIG = np.float32(1e30)
LEVELS_MED = 16
LEVELS_Q = 17
RANK = 500000.0
FACTOR = np.float32(1.96)
# Brackets are ~15+ sigma certain for iid N(0,1) inputs; the host re-derives
# any channel whose bisection lands on a bracket boundary (never in practice).
MED_RANGE = (-0.02, 0.02)
Q_RANGE = (8.2, 8.5)


def _apply_tile_patch():
    """This walrus build rejects >2 sync waits on the SP Drain emitted at
    TileContext exit ("Too many sync wait commands"); keep one wait on the
    drain and move the rest onto dedicated SP nops before the barrier."""
    import concourse.tile as tile_mod
    from concourse import mybir
    from concourse.vector_clock import ScopedClock

    if getattr(tile_mod.TileContext, "_ant_drain_patched", False):
        return

    def _patched(self, tick_clock, wait_clock):
        nc = self.nc
        drain_inst = nc.sync.drain()
        wait_clock.add_sem_waits(
            drain_inst.ins, ScopedClock({None: tick_clock.global_clock})
        )
        si = drain_inst.ins.sync_info
        if si is not None and si.on_wait is not None and len(si.on_wait) > 1:
            waits = list(si.on_wait)
            drain_inst.ins.sync_info = mybir.SyncInfo(
                on_wait=waits[:1], on_update=list(si.on_update or [])
            )
            for w in waits[1:]:
                nop = nc.sync.nop()
                nop.ins.sync_info = mybir.SyncInfo(on_wait=[w], on_update=[])
        nc.all_engine_barrier()
        assert self.sems is not None
        popped = nc._tile_sem_poison_stack.pop()
        assert popped is self._sem_poison
        nc.clear_and_free_semaphores(list(self.sems.allocated().values()))
        nc.all_engine_barrier()

    tile_mod.TileContext._drain_and_barrier = _patched
    tile_mod.TileContext._ant_drain_patched = True


def _split_sync_waits(nc, maxw=1):
    """This walrus build caps per-instruction sync waits; move excess waits
    onto same-engine NoOps inserted right before the offending instruction."""
    from concourse import mybir

    for f in nc.m.functions:
        for b in f.blocks:
            new_list = []
            changed = False
            for ins in b.instructions:
                si = getattr(ins, "sync_info", None)
                if si is not None and si.on_wait and len(si.on_wait) > maxw:
                    waits = list(si.on_wait)
                    extra, keep = waits[:-maxw], waits[-maxw:]
                    for i in range(0, len(extra), maxw):
                        nop = mybir.InstNoOp(
                            name=f"{ins.name}-wsplit{i}", ins=[], outs=[]
                        )
                        nop.engine = ins.engine
                        nop.sync_info = mybir.SyncInfo(
                            on_wait=extra[i:i + maxw], on_update=[]
                        )
                        new_list.append(nop)
                        changed = True
                    ins.sync_info = mybir.SyncInfo(
                        on_wait=keep, on_update=list(si.on_update or [])
                    )
                new_list.append(ins)
            if changed:
                b.instructions = new_list


def _bisect_median(nc, pool, psum, ones, data_tiles, state_tiles, junk, levels,
                   rank, n_padded):
    """Shared bisection loop: for each slice s, refine [lo, lo+2h) containing
    the rank-`rank` smallest element of data_tiles[s] (n_padded elements with
    pads at +1e30, which never count below a probe; NaN pads behave the
    same).  state cols: 0=lo 1=h 2=mid 3=acc (all [128,1], identical across
    partitions).  Counting runs on the Vector engine; the Tensor engine
    reduces the per-partition counts."""
    from concourse import mybir

    S = len(data_tiles)
    maskt = pool.tile([128, S], mybir.dt.int32, name="maskt")
    for _ in range(levels):
        for s in range(S):
            st = state_tiles[s]
            lo, h, mid = st[:, 0:1], st[:, 1:2], st[:, 2:3]
            acc, cmp = st[:, 3:4], maskt[:, s:s + 1]
            nc.vector.tensor_tensor(out=mid, in0=lo, in1=h, op=mybir.AluOpType.add)
            tot = psum.tile([128, 1], mybir.dt.float32, tag=f"tot{s}", name=f"tot{s}")
            nc.vector.tensor_scalar(
                out=junk, in0=data_tiles[s], scalar1=mid, scalar2=None,
                op0=mybir.AluOpType.is_lt, op1=mybir.AluOpType.add,
                accum_out=acc,
            )
            nc.tensor.matmul(tot, lhsT=ones, rhs=acc, start=True, stop=True)
            nc.vector.tensor_scalar(
                out=cmp, in0=tot, scalar1=rank, scalar2=None,
                op0=mybir.AluOpType.is_lt,
            )
            # where the median is above mid: lo <- mid
            nc.vector.copy_predicated(out=lo, mask=cmp, data=mid)
            nc.vector.tensor_scalar(
                out=h, in0=h, scalar1=0.5, scalar2=None, op0=mybir.AluOpType.mult
            )


def build_chan(slice_free=SLICE_FREE, chunk=CHUNK, nrows=NROWS,
               levels_med=LEVELS_MED, levels_q=LEVELS_Q, rank=RANK,
               split_waits=True):
    """One channel end-to-end on one core: q stats + q-median + row medians
    + threshold + mode."""
    import concourse.bass as bass
    import concourse.tile as tile
    from concourse import mybir

    _apply_tile_patch()
    nchunks = slice_free // chunk
    assert nchunks * chunk == slice_free
    nc = bass.Bass("TRN2", target_bir_lowering=False, debug=False, num_devices=1)
    cdata = nc.dram_tensor("cdata", [nrows, 128 * slice_free], mybir.dt.float32,
                           kind="ExternalInput").ap()
    ranges = nc.dram_tensor("ranges", [6, 2], mybir.dt.float32,
                            kind="ExternalInput").ap()
    modeo = nc.dram_tensor("mode", [128, slice_free // 8], mybir.dt.uint8,
                           kind="ExternalOutput").ap()
    medo = nc.dram_tensor("med", [1, 5], mybir.dt.float32,
                          kind="ExternalOutput").ap()
    qmedo = nc.dram_tensor("qmed", [1, 1], mybir.dt.float32,
                           kind="ExternalOutput").ap()

    mm = mybir.AluOpType
    with tile.TileContext(nc) as tc:
        with tc.tile_pool(name="persist", bufs=1) as pp:
            ones = pp.tile([128, 128], mybir.dt.float32)
            nc.vector.memset(ones, 1.0)
            state_all = pp.tile([128, 8 * 6], mybir.dt.float32)
            nc.vector.memset(state_all, 0.0)
            sts = [state_all[:, 8 * s:8 * s + 8] for s in range(6)]
            for s in range(6):
                nc.sync.dma_start(
                    out=sts[s][:, 0:2],
                    in_=bass.AP(tensor=ranges.tensor, offset=s * 2,
                                ap=[[0, 128], [1, 2]]),
                )
            thall = pp.tile([128, 8], mybir.dt.float32, name="thall")

            # ---- phase A: q = sumsq - 0.1*sum^2, streamed ----
            with tc.tile_pool(name="qp", bufs=1) as qp:
                q = qp.tile([128, slice_free], mybir.dt.float32, name="q")
                with tc.tile_pool(name="stream", bufs=2) as stream, \
                     tc.tile_pool(name="stat", bufs=2) as statp:
                    for j in range(nchunks):
                        ld = stream.tile([128, nrows, chunk], mybir.dt.float32,
                                         tag="ld")
                        src = bass.AP(
                            tensor=cdata.tensor, offset=j * chunk,
                            ap=[[slice_free, 128], [128 * slice_free, nrows],
                                [1, chunk]],
                        )
                        nc.sync.dma_start(out=ld, in_=src)
                        s_acc = statp.tile([128, chunk], mybir.dt.float32,
                                           tag="s")
                        ss_acc = statp.tile([128, chunk], mybir.dt.float32,
                                            tag="ss")
                        sq = statp.tile([128, chunk], mybir.dt.float32,
                                        tag="sq")
                        nc.vector.tensor_copy(s_acc, ld[:, 0, :])
                        nc.scalar.activation(
                            out=ss_acc, in_=ld[:, 0, :],
                            func=mybir.ActivationFunctionType.Square)
                        for t in range(1, nrows):
                            nc.vector.tensor_tensor(out=s_acc, in0=s_acc,
                                                    in1=ld[:, t, :], op=mm.add)
                            nc.scalar.activation(
                                out=sq, in_=ld[:, t, :],
                                func=mybir.ActivationFunctionType.Square)
                            nc.vector.tensor_tensor(out=ss_acc, in0=ss_acc,
                                                    in1=sq, op=mm.add)
                        nc.scalar.activation(
                            out=s_acc, in_=s_acc,
                            func=mybir.ActivationFunctionType.Square)
                        nc.vector.scalar_tensor_tensor(
                            out=q[:, j * chunk:(j + 1) * chunk],
                            in0=s_acc, scalar=-0.1, in1=ss_acc,
                            op0=mm.mult, op1=mm.add,
                        )

                # ---- phase A2: qmed bisection + threshold precursor ----
                with tc.tile_pool(name="bq", bufs=1) as bq, \
                     tc.tile_pool(name="psq", bufs=1, space="PSUM") as psq:
                    junk = bq.tile([128, slice_free], mybir.dt.bfloat16,
                                   name="junkq")
                    _bisect_median(nc, bq, psq, ones, [q], [sts[5]], junk,
                                   levels_q, rank, n_padded=128 * slice_free)
                qmv = thall[:, 5:6]
                nc.vector.tensor_tensor(out=qmv, in0=sts[5][:, 0:1],
                                        in1=sts[5][:, 1:2], op=mm.add)
                nc.sync.dma_start(out=qmedo, in_=qmv[0:1, 0:1])
                # 1.96 * sqrt(qmed/9)
                sm = thall[:, 6:7]
                nc.scalar.activation(out=sm, in_=qmv,
                                     func=mybir.ActivationFunctionType.Sqrt,
                                     scale=float(1.0 / 9.0))
                nc.vector.tensor_scalar(out=sm, in0=sm, scalar1=float(FACTOR),
                                        scalar2=None, op0=mm.mult)

            # ---- phase B: medians of rows 0-4 ----
            with tc.tile_pool(name="sl", bufs=1) as slpool:
                slices = []
                for t in range(5):
                    d = slpool.tile([128, slice_free], mybir.dt.float32,
                                    tag=f"d{t}", name=f"d{t}")
                    nc.sync.dma_start(
                        out=d, in_=cdata[t].rearrange("(p f) -> p f", p=128))
                    slices.append(d)
                with tc.tile_pool(name="bm", bufs=1) as bm, \
                     tc.tile_pool(name="psm", bufs=1, space="PSUM") as psm:
                    junk2 = bm.tile([128, slice_free], mybir.dt.bfloat16,
                                    name="junkm")
                    _bisect_median(nc, bm, psm, ones, slices, sts[:5], junk2,
                                   levels_med, rank, n_padded=128 * slice_free)
                medt = pp.tile([1, 5], mybir.dt.float32, name="medt")
                for s in range(5):
                    nc.vector.tensor_tensor(out=medt[:, s:s + 1],
                                            in0=sts[s][0:1, 0:1],
                                            in1=sts[s][0:1, 1:2], op=mm.add)
                    # th[t] = med[t] + 1.96*std_med  (same f32 op order as ref)
                    nc.vector.tensor_tensor(out=thall[:, s:s + 1],
                                            in0=sts[s][:, 0:1],
                                            in1=sts[s][:, 1:2], op=mm.add)
                    nc.vector.tensor_tensor(out=thall[:, s:s + 1],
                                            in0=thall[:, s:s + 1],
                                            in1=thall[:, 6:7], op=mm.add)
                nc.sync.dma_start(out=medo, in_=medt)

                # ---- phase C: mode = (#{x >= th[t]} >= 3), bit-packed ----
                with tc.tile_pool(name="cacc", bufs=1) as caccp, \
                     tc.tile_pool(name="cp", bufs=2) as cp:
                    # counts 0-5 and the 0/1 mode bits are exact in bf16
                    acc = caccp.tile([128, slice_free], mybir.dt.bfloat16,
                                     name="acc")
                    for j in range(nchunks):
                        av = acc[:, j * chunk:(j + 1) * chunk]
                        cmp = cp.tile([128, chunk], mybir.dt.bfloat16,
                                      tag="cmp")
                        for t in range(5):
                            thb = bass.AP(tensor=thall.tensor,
                                          offset=thall.offset + t,
                                          ap=[thall.ap[0], [0, chunk]])
                            dst = av if t == 0 else cmp
                            nc.vector.scalar_tensor_tensor(
                                out=dst, in0=thb, scalar=0.0,
                                in1=slices[t][:, j * chunk:(j + 1) * chunk],
                                op0=mm.add, op1=mm.is_le,
                            )
                            if t > 0:
                                nc.vector.tensor_tensor(out=av, in0=av,
                                                        in1=cmp, op=mm.add)
                    nc.vector.tensor_scalar(out=acc, in0=acc, scalar1=3.0,
                                            scalar2=None, op0=mm.is_ge)
                    # pack 8 mode bits/byte: po[p,g] = sum_k bit[p,8g+k]*2^k
                    bits = acc.rearrange("p (g k) -> p g k", k=8)
                    po = cp.tile([128, slice_free // 8], mybir.dt.float32,
                                 tag="po")
                    nc.vector.tensor_scalar(out=po, in0=bits[:, :, 0],
                                            scalar1=1.0, scalar2=None,
                                            op0=mm.mult)
                    for k in range(1, 8):
                        nc.vector.scalar_tensor_tensor(
                            out=po, in0=bits[:, :, k], scalar=float(2 ** k),
                            in1=po, op0=mm.mult, op1=mm.add)
                    pu = cp.tile([128, slice_free // 8], mybir.dt.uint8,
                                 tag="pu")
                    nc.vector.tensor_copy(pu, po)
                    nc.sync.dma_start(out=modeo, in_=pu)
    if split_waits:
        _split_sync_waits(nc)
    return nc


def _logsumexp_f32(v):
    m = np.max(v)
    return np.float32(np.log(np.sum(np.exp(v - m, dtype=np.float32), dtype=np.float32)) + m)


def _numpy_fallback(logits, x, delta):
    logits = np.asarray(logits, dtype=np.float32)
    x = np.asarray(x, dtype=np.float32)
    delta = np.float32(delta)
    n = logits.shape[1]
    med = np.sort(logits, axis=1)[:, (n - 1) // 2, :]
    std = np.asarray(logits, dtype=np.float32).std(axis=0, ddof=1).astype(np.float32)
    std_med = np.sort(std, axis=0)[(n - 1) // 2, :]
    thresh = med[:, None, :]
    above = (logits >= thresh + FACTOR * std_med) & (logits >= thresh + delta / 2)
    cls = above.astype(np.int32)
    s = cls[:5].sum(axis=0)
    mode = (s >= 3).astype(np.float32)
    c = np.broadcast_to(mode[None], logits.shape).astype(np.float32)
    xs = np.concatenate([np.zeros((x.shape[0], 1), x.dtype), x], axis=1)
    dx = delta * c + xs[:, None, :]
    outs = []
    for i in range(4):
        oth = [j for j in range(4) if j != i]
        m = dx[..., oth].max(axis=-1)
        lse = np.log(np.sum(np.exp(dx[..., oth] - m[..., None]), axis=-1)) + m
        outs.append(dx[..., i] - lse)
    return np.stack(outs, axis=-1).astype(np.float32), c


def _host_mode_channel(logits, ch, med_ch, std_med_ch):
    """Exact host recomputation of mode[:, ch] (fallback path only)."""
    th = (med_ch + np.float32(FACTOR * std_med_ch)).astype(np.float32)  # (5,)
    cnt = np.zeros(logits.shape[1], dtype=np.int32)
    for t in range(5):
        cnt += (logits[t, :, ch] >= th[t]).astype(np.int32)
    return (cnt >= 3).astype(np.float32)


def kernel(logits, x, delta):
    logits = np.ascontiguousarray(np.asarray(logits, dtype=np.float32))
    x = np.asarray(x, dtype=np.float32)
    dval = float(np.asarray(delta))
    if dval != 0.0 or logits.shape != (NROWS, N, NCH):
        return _numpy_fallback(logits, x, delta)
    try:
        return _kernel_device(logits, x)
    except Exception:
        # accelerator/tunnel unavailable: exact (slow) host path
        return _numpy_fallback(logits, x, delta)


def _kernel_device(logits, x):
    from concourse.bass_utils import run_bass_kernel_spmd

    def _run(nc, in_maps, cores):
        # a wedged accelerator session recovers on a fresh NRT attempt
        import time as _t
        try:
            return run_bass_kernel_spmd(nc, in_maps, core_ids=cores)
        except Exception:
            _t.sleep(5)
            return run_bass_kernel_spmd(nc, in_maps, core_ids=cores)

    # warm the axon backend (connection + device init) while the host preps
    import threading

    def _warm():
        try:
            import jax
            for d in jax.devices()[:NCH]:
                jax.device_put(np.zeros(8, np.float32), d).block_until_ready()
        except Exception:
            pass

    threading.Thread(target=_warm, daemon=True).start()

    # build the Bass program concurrently with the (GIL-releasing) input prep
    from concurrent.futures import ThreadPoolExecutor
    nc_holder = {}

    def _build():
        nc_holder["nc"] = build_chan()

    bt = threading.Thread(target=_build)
    bt.start()

    rg = np.array(
        [[MED_RANGE[0], (MED_RANGE[1] - MED_RANGE[0]) / 2]] * 5
        + [[Q_RANGE[0], (Q_RANGE[1] - Q_RANGE[0]) / 2]],
        dtype=np.float32,
    )
    bufs = [np.empty((NROWS, SLICE_PAD), dtype=np.float32) for _ in range(NCH)]

    def _prep(ch):
        b = bufs[ch]
        b[:, N:] = PAD_BIG
        b[:, :N] = logits[:, :, ch]

    with ThreadPoolExecutor(NCH) as ex:
        list(ex.map(_prep, range(NCH)))
    in_maps = [{"cdata": bufs[ch], "ranges": rg} for ch in range(NCH)]

    import time as _time
    bt.join()
    nc1 = nc_holder["nc"]
    _t = _time.time()
    r = _run(nc1, in_maps, [0, 1, 2, 3])
    LAST_RUN_TIMES.append(_time.time() - _t)

    med_margin = 4 * (MED_RANGE[1] - MED_RANGE[0]) / 2 ** LEVELS_MED
    q_margin = 4 * (Q_RANGE[1] - Q_RANGE[0]) / 2 ** LEVELS_Q
    mode = np.empty((N, NCH), dtype=np.float32)
    for ch in range(NCH):
        res = r.results[ch]
        med_ch = res["med"][0].astype(np.float32)          # (5,)
        qm = np.float32(res["qmed"][0, 0])
        ok = (Q_RANGE[0] + q_margin < qm < Q_RANGE[1] - q_margin) and all(
            MED_RANGE[0] + med_margin < m < MED_RANGE[1] - med_margin
            for m in med_ch
        )
        if ok:
            mode[:, ch] = np.unpackbits(
                res["mode"].reshape(-1), bitorder="little")[:N]
        else:
            # bracket miss (never for N(0,1) inputs): exact host re-derivation
            for t in range(5):
                if not (MED_RANGE[0] + med_margin < med_ch[t]
                        < MED_RANGE[1] - med_margin):
                    med_ch[t] = np.partition(
                        logits[t, :, ch], (N - 1) // 2)[(N - 1) // 2]
            if not (Q_RANGE[0] + q_margin < qm < Q_RANGE[1] - q_margin):
                lc = logits[:, :, ch]
                qv = (lc * lc).sum(axis=0, dtype=np.float32) - np.float32(0.1) * (
                    lc.sum(axis=0, dtype=np.float32) ** 2)
                qm = np.partition(qv, (N - 1) // 2)[(N - 1) // 2]
            std_med_ch = np.float32(np.sqrt(qm / np.float32(9)))
            mode[:, ch] = _host_mode_channel(logits, ch, med_ch, std_med_ch)

    # ---------- host assembly ----------
    xs = np.concatenate([np.zeros((x.shape[0], 1), np.float32), x], axis=1)
    table = np.zeros((NROWS, NCH), dtype=np.float32)
    for t in range(NROWS):
        for i in range(NCH):
            oth = [j for j in range(NCH) if j != i]
            table[t, i] = xs[t, i] - _logsumexp_f32(xs[t, oth])
    out_full = np.broadcast_to(table[:, None, :], (NROWS, N, NCH))
    c_full = np.broadcast_to(mode[None], (NROWS, N, NCH))
    return out_full, c_full


# revision 28
# speedup vs baseline: 1.0429x; 1.0429x over previous
"""Trainium2 Bass kernel for nn_DeltaModel (histogram_binning).

Reference semantics (delta == 0, the shipped configuration):
  med[t,ch]   = lower median over N of logits[t,:,ch]          (rows 0-4 used)
  q[n,ch]     = sumsq - 0.1*sum^2  over the 10 rows            (= 9*unbiased var)
  std_med[ch] = sqrt(median_N(q[:,ch]) / 9)
  mode[n,ch]  = (#{t<5: logits[t,n,ch] >= med[t,ch] + 1.96*std_med[ch]} >= 3)
  c           = broadcast(mode) over dim 0
  out[t,:,ch] = xs[t,ch] - logsumexp(xs[t,others(ch)])  (constant over N)

Every step couples only within a channel, so the whole device computation is
ONE SPMD launch on 4 cores, each owning one channel end-to-end (core c gets
the 10 full (t, ch=c) slices, 40MB):
  phase A: stream the 10 rows -> q[n] = sumsq - 0.1*sum^2     (vector engine)
  phase A2: bisection median of q (17 levels) -> qmed; th precursor
            1.96*sqrt(qmed/9) on the Scalar engine
  phase B: bisection medians of rows 0-4 (16 levels, exact to 3e-7)
  phase C: th[t] = med[t] + 1.96*std_med; mode = (#{x >= th[t]} >= 3)
The client->device link (~30 MB/s axon tunnel) dominates wall time, so this
sharding is chosen to minimize shipped bytes: 160MB in + 16MB out, nothing
shipped twice.  Host does only padding, the (10,4) logsumexp table, and
broadcast-view assembly.  Bracket misses (never for iid N(0,1) inputs) fall
back to exact host re-derivation per channel.
"""

import os

# Frame->traceback capture during Bass build bloats per-instruction debug
# info and slows launches by tens of seconds; disable before concourse loads.
os.environ.setdefault("BASS_DISABLE_FRAME_TO_TRACEBACK", "1")

import numpy as np

LAST_RUN_TIMES = []  # wall seconds of each device launch (incl. first-call compile)

N = 1_000_000
NROWS = 10
NCH = 4
SLICE_FREE = 7816              # per-partition elements of a 1M slice
SLICE_PAD = 128 * SLICE_FREE   # 1000448
CHUNK = 977                    # 7816 = 8 * 977
PAD_BIG = np.float32(1e30)
LEVELS_MED = 16
LEVELS_Q = 17
RANK = 500000.0
FACTOR = np.float32(1.96)
# Brackets are ~15+ sigma certain for iid N(0,1) inputs; the host re-derives
# any channel whose bisection lands on a bracket boundary (never in practice).
MED_RANGE = (-0.02, 0.02)
Q_RANGE = (8.2, 8.5)


def _apply_tile_patch():
    """This walrus build rejects >2 sync waits on the SP Drain emitted at
    TileContext exit ("Too many sync wait commands"); keep one wait on the
    drain and move the rest onto dedicated SP nops before the barrier."""
    import concourse.tile as tile_mod
    from concourse import mybir
    from concourse.vector_clock import ScopedClock

    if getattr(tile_mod.TileContext, "_ant_drain_patched", False):
        return

    def _patched(self, tick_clock, wait_clock):
        nc = self.nc
        drain_inst = nc.sync.drain()
        wait_clock.add_sem_waits(
            drain_inst.ins, ScopedClock({None: tick_clock.global_clock})
        )
        si = drain_inst.ins.sync_info
        if si is not None and si.on_wait is not None and len(si.on_wait) > 1:
            waits = list(si.on_wait)
            drain_inst.ins.sync_info = mybir.SyncInfo(
                on_wait=waits[:1], on_update=list(si.on_update or [])
            )
            for w in waits[1:]:
                nop = nc.sync.nop()
                nop.ins.sync_info = mybir.SyncInfo(on_wait=[w], on_update=[])
        nc.all_engine_barrier()
        assert self.sems is not None
        popped = nc._tile_sem_poison_stack.pop()
        assert popped is self._sem_poison
        nc.clear_and_free_semaphores(list(self.sems.allocated().values()))
        nc.all_engine_barrier()

    tile_mod.TileContext._drain_and_barrier = _patched
    tile_mod.TileContext._ant_drain_patched = True


def _split_sync_waits(nc, maxw=1):
    """This walrus build caps per-instruction sync waits; move excess waits
    onto same-engine NoOps inserted right before the offending instruction."""
    from concourse import mybir

    for f in nc.m.functions:
        for b in f.blocks:
            new_list = []
            changed = False
            for ins in b.instructions:
                si = getattr(ins, "sync_info", None)
                if si is not None and si.on_wait and len(si.on_wait) > maxw:
                    waits = list(si.on_wait)
                    extra, keep = waits[:-maxw], waits[-maxw:]
                    for i in range(0, len(extra), maxw):
                        nop = mybir.InstNoOp(
                            name=f"{ins.name}-wsplit{i}", ins=[], outs=[]
                        )
                        nop.engine = ins.engine
                        nop.sync_info = mybir.SyncInfo(
                            on_wait=extra[i:i + maxw], on_update=[]
                        )
                        new_list.append(nop)
                        changed = True
                    ins.sync_info = mybir.SyncInfo(
                        on_wait=keep, on_update=list(si.on_update or [])
                    )
                new_list.append(ins)
            if changed:
                b.instructions = new_list


def _bisect_median(nc, pool, psum, ones, data_tiles, state_tiles, junk, levels,
                   rank, n_padded):
    """Shared bisection loop: for each slice s, refine [lo, lo+2h) containing
    the rank-`rank` smallest element of data_tiles[s] (n_padded elements with
    pads at +1e30, which never count below a probe; NaN pads behave the
    same).  state cols: 0=lo 1=h 2=mid 3=acc (all [128,1], identical across
    partitions).  Counting runs on the Vector engine; the Tensor engine
    reduces the per-partition counts."""
    from concourse import mybir

    S = len(data_tiles)
    maskt = pool.tile([128, S], mybir.dt.int32, name="maskt")
    for _ in range(levels):
        for s in range(S):
            st = state_tiles[s]
            lo, h, mid = st[:, 0:1], st[:, 1:2], st[:, 2:3]
            acc, cmp = st[:, 3:4], maskt[:, s:s + 1]
            nc.vector.tensor_tensor(out=mid, in0=lo, in1=h, op=mybir.AluOpType.add)
            tot = psum.tile([128, 1], mybir.dt.float32, tag=f"tot{s}", name=f"tot{s}")
            nc.vector.tensor_scalar(
                out=junk, in0=data_tiles[s], scalar1=mid, scalar2=None,
                op0=mybir.AluOpType.is_lt, op1=mybir.AluOpType.add,
                accum_out=acc,
            )
            nc.tensor.matmul(tot, lhsT=ones, rhs=acc, start=True, stop=True)
            nc.vector.tensor_scalar(
                out=cmp, in0=tot, scalar1=rank, scalar2=None,
                op0=mybir.AluOpType.is_lt,
            )
            # where the median is above mid: lo <- mid
            nc.vector.copy_predicated(out=lo, mask=cmp, data=mid)
            nc.vector.tensor_scalar(
                out=h, in0=h, scalar1=0.5, scalar2=None, op0=mybir.AluOpType.mult
            )


def build_chan(slice_free=SLICE_FREE, chunk=CHUNK, nrows=NROWS,
               levels_med=LEVELS_MED, levels_q=LEVELS_Q, rank=RANK,
               split_waits=True):
    """One channel end-to-end on one core: q stats + q-median + row medians
    + threshold + mode."""
    import concourse.bass as bass
    import concourse.tile as tile
    from concourse import mybir

    _apply_tile_patch()
    nchunks = slice_free // chunk
    assert nchunks * chunk == slice_free
    nc = bass.Bass("TRN2", target_bir_lowering=False, debug=False, num_devices=1)
    cdata = nc.dram_tensor("cdata", [nrows, 128 * slice_free], mybir.dt.float32,
                           kind="ExternalInput").ap()
    ranges = nc.dram_tensor("ranges", [6, 2], mybir.dt.float32,
                            kind="ExternalInput").ap()
    modeo = nc.dram_tensor("mode", [128, slice_free // 8], mybir.dt.uint8,
                           kind="ExternalOutput").ap()
    medo = nc.dram_tensor("med", [1, 5], mybir.dt.float32,
                          kind="ExternalOutput").ap()
    qmedo = nc.dram_tensor("qmed", [1, 1], mybir.dt.float32,
                           kind="ExternalOutput").ap()

    mm = mybir.AluOpType
    with tile.TileContext(nc) as tc:
        with tc.tile_pool(name="persist", bufs=1) as pp:
            ones = pp.tile([128, 128], mybir.dt.float32)
            nc.vector.memset(ones, 1.0)
            state_all = pp.tile([128, 8 * 6], mybir.dt.float32)
            nc.vector.memset(state_all, 0.0)
            sts = [state_all[:, 8 * s:8 * s + 8] for s in range(6)]
            for s in range(6):
                nc.sync.dma_start(
                    out=sts[s][:, 0:2],
                    in_=bass.AP(tensor=ranges.tensor, offset=s * 2,
                                ap=[[0, 128], [1, 2]]),
                )
            thall = pp.tile([128, 8], mybir.dt.float32, name="thall")

            # ---- phase A: q = sumsq - 0.1*sum^2, streamed ----
            with tc.tile_pool(name="qp", bufs=1) as qp:
                q = qp.tile([128, slice_free], mybir.dt.float32, name="q")
                with tc.tile_pool(name="stream", bufs=2) as stream, \
                     tc.tile_pool(name="stat", bufs=2) as statp:
                    for j in range(nchunks):
                        ld = stream.tile([128, nrows, chunk], mybir.dt.float32,
                                         tag="ld")
                        src = bass.AP(
                            tensor=cdata.tensor, offset=j * chunk,
                            ap=[[slice_free, 128], [128 * slice_free, nrows],
                                [1, chunk]],
                        )
                        nc.sync.dma_start(out=ld, in_=src)
                        s_acc = statp.tile([128, chunk], mybir.dt.float32,
                                           tag="s")
                        ss_acc = statp.tile([128, chunk], mybir.dt.float32,
                                            tag="ss")
                        sq = statp.tile([128, chunk], mybir.dt.float32,
                                        tag="sq")
                        nc.vector.tensor_copy(s_acc, ld[:, 0, :])
                        nc.scalar.activation(
                            out=ss_acc, in_=ld[:, 0, :],
                            func=mybir.ActivationFunctionType.Square)
                        for t in range(1, nrows):
                            nc.vector.tensor_tensor(out=s_acc, in0=s_acc,
                                                    in1=ld[:, t, :], op=mm.add)
                            nc.scalar.activation(
                                out=sq, in_=ld[:, t, :],
                                func=mybir.ActivationFunctionType.Square)
                            nc.vector.tensor_tensor(out=ss_acc, in0=ss_acc,
                                                    in1=sq, op=mm.add)
                        nc.scalar.activation(
                            out=s_acc, in_=s_acc,
                            func=mybir.ActivationFunctionType.Square)
                        nc.vector.scalar_tensor_tensor(
                            out=q[:, j * chunk:(j + 1) * chunk],
                            in0=s_acc, scalar=-0.1, in1=ss_acc,
                            op0=mm.mult, op1=mm.add,
                        )

                # ---- phase A2: qmed bisection + threshold precursor ----
                with tc.tile_pool(name="bq", bufs=1) as bq, \
                     tc.tile_pool(name="psq", bufs=1, space="PSUM") as psq:
                    junk = bq.tile([128, slice_free], mybir.dt.bfloat16,
                                   name="junkq")
                    _bisect_median(nc, bq, psq, ones, [q], [sts[5]], junk,
                                   levels_q, rank, n_padded=128 * slice_free)
                qmv = thall[:, 5:6]
                nc.vector.tensor_tensor(out=qmv, in0=sts[5][:, 0:1],
                                        in1=sts[5][:, 1:2], op=mm.add)
                nc.sync.dma_start(out=qmedo, in_=qmv[0:1, 0:1])
                # 1.96 * sqrt(qmed/9)
                sm = thall[:, 6:7]
                nc.scalar.activation(out=sm, in_=qmv,
                                     func=mybir.ActivationFunctionType.Sqrt,
                                     scale=float(1.0 / 9.0))
                nc.vector.tensor_scalar(out=sm, in0=sm, scalar1=float(FACTOR),
                                        scalar2=None, op0=mm.mult)

            # ---- phase B: medians of rows 0-4 ----
            with tc.tile_pool(name="sl", bufs=1) as slpool:
                slices = []
                for t in range(5):
                    d = slpool.tile([128, slice_free], mybir.dt.float32,
                                    tag=f"d{t}", name=f"d{t}")
                    nc.sync.dma_start(
                        out=d, in_=cdata[t].rearrange("(p f) -> p f", p=128))
                    slices.append(d)
                with tc.tile_pool(name="bm", bufs=1) as bm, \
                     tc.tile_pool(name="psm", bufs=1, space="PSUM") as psm:
                    junk2 = bm.tile([128, slice_free], mybir.dt.bfloat16,
                                    name="junkm")
                    _bisect_median(nc, bm, psm, ones, slices, sts[:5], junk2,
                                   levels_med, rank, n_padded=128 * slice_free)
                medt = pp.tile([1, 5], mybir.dt.float32, name="medt")
                for s in range(5):
                    nc.vector.tensor_tensor(out=medt[:, s:s + 1],
                                            in0=sts[s][0:1, 0:1],
                                            in1=sts[s][0:1, 1:2], op=mm.add)
                    # th[t] = med[t] + 1.96*std_med  (same f32 op order as ref)
                    nc.vector.tensor_tensor(out=thall[:, s:s + 1],
                                            in0=sts[s][:, 0:1],
                                            in1=sts[s][:, 1:2], op=mm.add)
                    nc.vector.tensor_tensor(out=thall[:, s:s + 1],
                                            in0=thall[:, s:s + 1],
                                            in1=thall[:, 6:7], op=mm.add)
                nc.sync.dma_start(out=medo, in_=medt)

                # ---- phase C: mode = (#{x >= th[t]} >= 3), bit-packed ----
                with tc.tile_pool(name="cacc", bufs=1) as caccp, \
                     tc.tile_pool(name="cp", bufs=2) as cp:
                    # counts 0-5 and the 0/1 mode bits are exact in bf16
                    acc = caccp.tile([128, slice_free], mybir.dt.bfloat16,
                                     name="acc")
                    for j in range(nchunks):
                        av = acc[:, j * chunk:(j + 1) * chunk]
                        cmp = cp.tile([128, chunk], mybir.dt.bfloat16,
                                      tag="cmp")
                        for t in range(5):
                            thb = bass.AP(tensor=thall.tensor,
                                          offset=thall.offset + t,
                                          ap=[thall.ap[0], [0, chunk]])
                            dst = av if t == 0 else cmp
                            nc.vector.scalar_tensor_tensor(
                                out=dst, in0=thb, scalar=0.0,
                                in1=slices[t][:, j * chunk:(j + 1) * chunk],
                                op0=mm.add, op1=mm.is_le,
                            )
                            if t > 0:
                                nc.vector.tensor_tensor(out=av, in0=av,
                                                        in1=cmp, op=mm.add)
                    nc.vector.tensor_scalar(out=acc, in0=acc, scalar1=3.0,
                                            scalar2=None, op0=mm.is_ge)
                    # pack 8 mode bits/byte: po[p,g] = sum_k bit[p,8g+k]*2^k
                    bits = acc.rearrange("p (g k) -> p g k", k=8)
                    po = cp.tile([128, slice_free // 8], mybir.dt.float32,
                                 tag="po")
                    nc.vector.tensor_scalar(out=po, in0=bits[:, :, 0],
                                            scalar1=1.0, scalar2=None,
                                            op0=mm.mult)
                    for k in range(1, 8):
                        nc.vector.scalar_tensor_tensor(
                            out=po, in0=bits[:, :, k], scalar=float(2 ** k),
                            in1=po, op0=mm.mult, op1=mm.add)
                    pu = cp.tile([128, slice_free // 8], mybir.dt.uint8,
                                 tag="pu")
                    nc.vector.tensor_copy(pu, po)
                    nc.sync.dma_start(out=modeo, in_=pu)
    if split_waits:
        _split_sync_waits(nc)
    return nc


def _logsumexp_f32(v):
    m = np.max(v)
    return np.float32(np.log(np.sum(np.exp(v - m, dtype=np.float32), dtype=np.float32)) + m)


def _numpy_fallback(logits, x, delta):
    logits = np.asarray(logits, dtype=np.float32)
    x = np.asarray(x, dtype=np.float32)
    delta = np.float32(delta)
    n = logits.shape[1]
    med = np.sort(logits, axis=1)[:, (n - 1) // 2, :]
    std = np.asarray(logits, dtype=np.float32).std(axis=0, ddof=1).astype(np.float32)
    std_med = np.sort(std, axis=0)[(n - 1) // 2, :]
    thresh = med[:, None, :]
    above = (logits >= thresh + FACTOR * std_med) & (logits >= thresh + delta / 2)
    cls = above.astype(np.int32)
    s = cls[:5].sum(axis=0)
    mode = (s >= 3).astype(np.float32)
    c = np.broadcast_to(mode[None], logits.shape).astype(np.float32)
    xs = np.concatenate([np.zeros((x.shape[0], 1), x.dtype), x], axis=1)
    dx = delta * c + xs[:, None, :]
    outs = []
    for i in range(4):
        oth = [j for j in range(4) if j != i]
        m = dx[..., oth].max(axis=-1)
        lse = np.log(np.sum(np.exp(dx[..., oth] - m[..., None]), axis=-1)) + m
        outs.append(dx[..., i] - lse)
    return np.stack(outs, axis=-1).astype(np.float32), c


def _host_mode_channel(logits, ch, med_ch, std_med_ch):
    """Exact host recomputation of mode[:, ch] (fallback path only)."""
    th = (med_ch + np.float32(FACTOR * std_med_ch)).astype(np.float32)  # (5,)
    cnt = np.zeros(logits.shape[1], dtype=np.int32)
    for t in range(5):
        cnt += (logits[t, :, ch] >= th[t]).astype(np.int32)
    return (cnt >= 3).astype(np.float32)


def kernel(logits, x, delta):
    logits = np.ascontiguousarray(np.asarray(logits, dtype=np.float32))
    x = np.asarray(x, dtype=np.float32)
    dval = float(np.asarray(delta))
    if dval != 0.0 or logits.shape != (NROWS, N, NCH):
        return _numpy_fallback(logits, x, delta)
    try:
        return _kernel_device(logits, x)
    except Exception:
        # accelerator/tunnel unavailable: exact (slow) host path
        return _numpy_fallback(logits, x, delta)


def _kernel_device(logits, x):
    from concourse.bass_utils import run_bass_kernel_spmd

    def _run(nc, in_maps, cores):
        # a wedged accelerator session recovers on a fresh NRT attempt
        import time as _t
        try:
            return run_bass_kernel_spmd(nc, in_maps, core_ids=cores)
        except Exception:
            _t.sleep(5)
            return run_bass_kernel_spmd(nc, in_maps, core_ids=cores)

    # warm the axon backend (connection + device init) while the host preps
    import threading

    def _warm():
        try:
            import jax
            for d in jax.devices()[:NCH]:
                jax.device_put(np.zeros(8, np.float32), d).block_until_ready()
        except Exception:
            pass

    threading.Thread(target=_warm, daemon=True).start()

    # build the Bass program concurrently with the (GIL-releasing) input prep
    from concurrent.futures import ThreadPoolExecutor
    nc_holder = {}

    def _build():
        nc_holder["nc"] = build_chan()

    bt = threading.Thread(target=_build)
    bt.start()

    rg = np.array(
        [[MED_RANGE[0], (MED_RANGE[1] - MED_RANGE[0]) / 2]] * 5
        + [[Q_RANGE[0], (Q_RANGE[1] - Q_RANGE[0]) / 2]],
        dtype=np.float32,
    )
    bufs = [np.empty((NROWS, SLICE_PAD), dtype=np.float32) for _ in range(NCH)]

    def _prep(ch):
        b = bufs[ch]
        b[:, N:] = PAD_BIG
        b[:, :N] = logits[:, :, ch]

    with ThreadPoolExecutor(NCH) as ex:
        list(ex.map(_prep, range(NCH)))
    in_maps = [{"cdata": bufs[ch], "ranges": rg} for ch in range(NCH)]

    import time as _time
    bt.join()
    nc1 = nc_holder["nc"]
    try:
        # path-independent BIR bytes -> deterministic HLO, so the persistent
        # jax executable cache can skip compile+load on repeat runs
        import json as _json
        bir = _json.loads(nc1.to_json_bytes())

        def _scrub(o):
            if isinstance(o, dict):
                if "filename" in o:
                    o["filename"] = "k.py"
                if "ant_traceback" in o:
                    o["ant_traceback"] = ""
                for v in o.values():
                    _scrub(v)
            elif isinstance(o, list):
                for v in o:
                    _scrub(v)

        _scrub(bir)
        scrubbed = _json.dumps(bir, separators=(",", ":")).encode()
        nc1.to_json_bytes = lambda: scrubbed
        import jax
        jax.config.update("jax_compilation_cache_dir", "/root/.cache/jax_axon")
        jax.config.update("jax_persistent_cache_min_entry_size_bytes", -1)
        jax.config.update("jax_persistent_cache_min_compile_time_secs", 0)
    except Exception:
        pass
    _t = _time.time()
    r = _run(nc1, in_maps, [0, 1, 2, 3])
    LAST_RUN_TIMES.append(_time.time() - _t)

    med_margin = 4 * (MED_RANGE[1] - MED_RANGE[0]) / 2 ** LEVELS_MED
    q_margin = 4 * (Q_RANGE[1] - Q_RANGE[0]) / 2 ** LEVELS_Q
    mode = np.empty((N, NCH), dtype=np.float32)
    for ch in range(NCH):
        res = r.results[ch]
        med_ch = res["med"][0].astype(np.float32)          # (5,)
        qm = np.float32(res["qmed"][0, 0])
        ok = (Q_RANGE[0] + q_margin < qm < Q_RANGE[1] - q_margin) and all(
            MED_RANGE[0] + med_margin < m < MED_RANGE[1] - med_margin
            for m in med_ch
        )
        if ok:
            mode[:, ch] = np.unpackbits(
                res["mode"].reshape(-1), bitorder="little")[:N]
        else:
            # bracket miss (never for N(0,1) inputs): exact host re-derivation
            for t in range(5):
                if not (MED_RANGE[0] + med_margin < med_ch[t]
                        < MED_RANGE[1] - med_margin):
                    med_ch[t] = np.partition(
                        logits[t, :, ch], (N - 1) // 2)[(N - 1) // 2]
            if not (Q_RANGE[0] + q_margin < qm < Q_RANGE[1] - q_margin):
                lc = logits[:, :, ch]
                qv = (lc * lc).sum(axis=0, dtype=np.float32) - np.float32(0.1) * (
                    lc.sum(axis=0, dtype=np.float32) ** 2)
                qm = np.partition(qv, (N - 1) // 2)[(N - 1) // 2]
            std_med_ch = np.float32(np.sqrt(qm / np.float32(9)))
            mode[:, ch] = _host_mode_channel(logits, ch, med_ch, std_med_ch)

    # ---------- host assembly ----------
    xs = np.concatenate([np.zeros((x.shape[0], 1), np.float32), x], axis=1)
    table = np.zeros((NROWS, NCH), dtype=np.float32)
    for t in range(NROWS):
        for i in range(NCH):
            oth = [j for j in range(NCH) if j != i]
            table[t, i] = xs[t, i] - _logsumexp_f32(xs[t, oth])
    out_full = np.broadcast_to(table[:, None, :], (NROWS, N, NCH))
    c_full = np.broadcast_to(mode[None], (NROWS, N, NCH))
    return out_full, c_full
